# revision 9
# baseline (speedup 1.0000x reference)
"""Trainium2 Bass kernel for the Jordan-RNN problem.

Math (reference):
    xh = emb @ Whx.T + bhx                      # [B,T,H] time-parallel
    scan t: h = tanh(xh_t + y @ Wyh.T + byh); y = h @ Wout.T + bout
    outputs: logits[:, t] = y_t ; last = y_T

Reformulation used here (y eliminated from the serial chain):
    W2 = Wyh @ Wout ; c2 = bout @ Wyh.T + byh
    xh2_t = emb_t @ Whx.T + bhx + c2            (phase 1, big GEMM)
    xh2_0 += y0 @ Wyh.T + byh - c2              (one-time device injection)
    h_t = tanh(xh2_t + h_{t-1} @ W2.T), h_{-1} = 0   (phase 2, serial)
    logits_t = h_t @ Wout.T + bout              (phase 3, big GEMM)

Sharding: data-parallel over batch across 8 cores (B_local = 8); weights
replicated; the T-recurrence stays local per core. SPMD: one NEFF, 8 cores.

Layouts (per core):
    emb      [BL*T, D]    natural
    xh2_d    [4, 128, BL, T]  (h-outer j, h-inner p, b, t)  == xh2^T tiles
    hall_d   [4, 128, BL, T]  same for h
    h in SBUF: [128, 4(j), 8(b)] per step -> directly the next step's matmul
    rhs (contraction dim H on partitions). No transposes in the recurrence.
"""

import os
import sys

sys.path.insert(0, "/opt/trn_rl_repo")

import numpy as np

import concourse.bass as bass
import concourse.mybir as mybir
import concourse.tile as tile
from concourse import bacc
from concourse.bass import ds, ts
from concourse.bass_utils import run_bass_kernel_spmd
from concourse.masks import make_identity

F32 = mybir.dt.float32
P = 128
NCORES = 8
B, T_FULL, D, H, O = 64, 2048, 512, 512, 512
BL = B // NCORES  # 8
HJ = H // P       # 4 h-tiles
KD = D // P       # 4 d-tiles
U = 64            # steps per chunk
CHUNKS_PER_ITER = 2



def _tctile(tc, shape, dtype, **kw):
    t, _free = tc.tile(shape, dtype, **kw)
    return t

def _build(T: int):
    """Emit the full 3-phase kernel for sequence length T. Returns nc."""
    BT = BL * T
    nc = bacc.Bacc("TRN2", target_bir_lowering=False, debug=False)

    # ---- I/O ----
    emb = nc.dram_tensor("emb", [BT, D], F32, kind="ExternalInput").ap()
    y0t = nc.dram_tensor("y0t", [P, HJ, BL], F32, kind="ExternalInput").ap()
    w2t = nc.dram_tensor("w2t", [P, HJ, H], F32, kind="ExternalInput").ap()
    wyht = nc.dram_tensor("wyht", [P, HJ, H], F32, kind="ExternalInput").ap()
    whxt = nc.dram_tensor("whxt", [P, KD, H], F32, kind="ExternalInput").ap()
    woutt = nc.dram_tensor("woutt", [P, HJ, O], F32, kind="ExternalInput").ap()
    xhb = nc.dram_tensor("xhb", [P, HJ], F32, kind="ExternalInput").ap()
    dc = nc.dram_tensor("dc", [P, HJ, BL], F32, kind="ExternalInput").ap()
    boutr = nc.dram_tensor("boutr", [P, O], F32, kind="ExternalInput").ap()
    logits = nc.dram_tensor("logits", [BL, T, O], F32, kind="ExternalOutput").ap()

    with tile.TileContext(nc) as tc, \
         tc.tile_pool(name="cst", bufs=1) as cst:
        # ---- persistent SBUF constants ----
        w2t_sb = cst.tile([P, HJ, H], F32, tag="w2t_sb")
        wyht_sb = cst.tile([P, HJ, H], F32, tag="wyht_sb")
        whxt_sb = cst.tile([P, KD, H], F32, tag="whxt_sb")
        woutt_sb = cst.tile([P, HJ, O], F32, tag="woutt_sb")
        y0t_sb = cst.tile([P, HJ, BL], F32, tag="y0t_sb")
        xhb_sb = cst.tile([P, HJ], F32, tag="xhb_sb")
        dc_sb = cst.tile([P, HJ, BL], F32, tag="dc_sb")
        bout_sb = cst.tile([P, O], F32, tag="bout_sb")
        ident = cst.tile([P, P], F32, tag="ident")

        nc.sync.dma_start(w2t_sb, w2t)
        nc.sync.dma_start(wyht_sb, wyht)
        nc.sync.dma_start(whxt_sb, whxt)
        nc.sync.dma_start(woutt_sb, woutt)
        nc.sync.dma_start(y0t_sb, y0t)
        nc.sync.dma_start(xhb_sb, xhb)
        nc.sync.dma_start(dc_sb, dc)
        nc.sync.dma_start(bout_sb, boutr)
        make_identity(nc, ident)

        # ---- DRAM scratch (Tile-tracked) ----
        xh2_d = _tctile(tc, [P, HJ, BL, T], F32, space="DRAM", name="xh2_d")
        hall_d = _tctile(tc, [P, HJ, BL, T], F32, space="DRAM", name="hall_d")

        # ================= Phase 1: xh2 = emb @ Whx.T + (bhx + c2) ========
        CS = min(512, T)   # bt-chunk size (within one b)
        SN = CS // P       # row-subtiles per chunk
        NCH = BT // CS
        TCH = T // CS      # t-chunks per b
        with tc.tile_pool(name="p1", bufs=3) as p1, \
             tc.tile_pool(name="p1pt", bufs=2, space="PSUM") as p1pt, \
             tc.tile_pool(name="p1pg", bufs=2, space="PSUM") as p1pg:
            for c in range(NCH):
                b, t0 = c // TCH, (c % TCH) * CS
                emb_nat = p1.tile([P, SN, 512], F32, tag="embnat")
                nc.sync.dma_start(
                    emb_nat,
                    emb[c * CS:(c + 1) * CS, :].rearrange(
                        "(s p) d -> p s d", p=P),
                )
                embT = p1.tile([P, KD, CS], F32, tag="embt")
                for kd in range(KD):
                    for s in range(SN):
                        pt = p1pt.tile([P, P], F32, tag="pt")
                        nc.tensor.transpose(
                            pt, emb_nat[:, s, kd * P:(kd + 1) * P], ident)
                        nc.vector.tensor_copy(
                            embT[:, kd, s * P:(s + 1) * P], pt)
                for j in range(HJ):
                    pg = p1pg.tile([P, CS], F32, tag="pg")
                    for kd in range(KD):
                        nc.tensor.matmul(
                            pg, whxt_sb[:, kd, ts(j, P)], embT[:, kd, :],
                            start=(kd == 0), stop=(kd == KD - 1))
                    xo = p1.tile([P, CS], F32, tag="xo")
                    nc.scalar.activation(
                        xo, pg, mybir.ActivationFunctionType.Identity,
                        bias=xhb_sb[:, j:j + 1])
                    nc.sync.dma_start(xh2_d[:, j, b, t0:t0 + CS], xo)

            # ---- inject Delta0 = y0 @ Wyh.T + byh - c2 into xh2[:, 0] ----
            pd = p1pg.tile([P, HJ * BL], F32, tag="pg0")
            for j in range(HJ):
                for k in range(HJ):
                    nc.tensor.matmul(
                        pd[:, ts(j, BL)], wyht_sb[:, k, ts(j, P)],
                        y0t_sb[:, k, :],
                        start=(j == 0 and k == 0),
                        stop=(j == HJ - 1 and k == HJ - 1))
            dsb = p1.tile([P, HJ, BL], F32, tag="dsb")
            nc.vector.tensor_add(
                dsb, pd.rearrange("p (j b) -> p j b", j=HJ), dc_sb)
            with nc.allow_non_contiguous_dma(reason="one-time 16KB inject"):
                nc.gpsimd.dma_start(
                    xh2_d[:, :, :, 0], dsb, accum_op=mybir.AluOpType.add)

        # ================= Phase 2: the T-step recurrence =================
        STEP = CHUNKS_PER_ITER * U
        with tc.tile_pool(name="rec", bufs=1) as rec, \
             tc.tile_pool(name="rpz", bufs=2, space="PSUM") as rpz:
            xh_t = [rec.tile([P, HJ, BL, U], F32, name=f"xh_{i}",
                             tag=f"xh_{i}") for i in range(CHUNKS_PER_ITER)]
            h_t = [rec.tile([P, HJ, BL, U], F32, name=f"h_{i}",
                            tag=f"h_{i}") for i in range(CHUNKS_PER_ITER)]
            nc.vector.memset(h_t[-1], 0.0)  # h_{-1} = 0 seed (last slot used)

            with tc.For_i(0, T, STEP,
                          hint_engines=(mybir.EngineType.PE,)) as iv:
                for half in range(CHUNKS_PER_ITER):
                    xh_c = xh_t[half]
                    h_c = h_t[half]
                    h_p = h_t[half - 1]
                    base = iv + half * U
                    nc.sync.dma_start(
                        xh_c, xh2_d[:, :, :, ds(base, U)])
                    for u in range(U):
                        hp, up = (h_c, u - 1) if u > 0 else (h_p, U - 1)
                        pz = rpz.tile([P, HJ * BL], F32, tag="pz")
                        for j in range(HJ):
                            for k in range(HJ):
                                nc.tensor.matmul(
                                    pz[:, ts(j, BL)],
                                    w2t_sb[:, k, ts(j, P)],
                                    hp[:, k, :, up],
                                    start=(j == 0 and k == 0),
                                    stop=(j == HJ - 1 and k == HJ - 1))
                        us = rec.tile([P, HJ, BL], F32, tag="us", bufs=2)
                        nc.vector.tensor_add(
                            us, pz.rearrange("p (j b) -> p j b", j=HJ),
                            xh_c[:, :, :, u])
                        nc.scalar.activation(
                            h_c[:, :, :, u], us,
                            mybir.ActivationFunctionType.Tanh)
                    nc.sync.dma_start(
                        hall_d[:, :, :, ds(base, U)], h_c)

        # ================= Phase 3: logits = Hall @ Wout.T + bout =========
        TT = T // P
        with tc.tile_pool(name="p3", bufs=8) as p3, \
             tc.tile_pool(name="p3pg", bufs=2, space="PSUM") as p3pg, \
             tc.tile_pool(name="p3o", bufs=3) as p3o:
            for b in range(BL):
                for tt in range(TT):
                    pg = p3pg.tile([P, O], F32, tag="p3g")
                    for j in range(HJ):
                        lt = p3.tile([P, P], F32, tag="lt")
                        nc.sync.dma_start(lt, hall_d[:, j, b, ts(tt, P)])
                        nc.tensor.matmul(
                            pg, lt, woutt_sb[:, j, :],
                            start=(j == 0), stop=(j == HJ - 1))
                    ob = p3o.tile([P, O], F32, tag="ob")
                    nc.vector.tensor_add(ob, pg, bout_sb)
                    nc.sync.dma_start(logits[b, ts(tt, P), :], ob)

    nc.compile()
    return nc


_CACHE: dict = {}


def _get_nc(T: int):
    if T not in _CACHE:
        _CACHE[T] = _build(T)
    return _CACHE[T]


def _pack_kmaj(w_t: np.ndarray) -> np.ndarray:
    """[K, M] (K = contraction, multiple of 128) -> [128, K//128, M]."""
    K, M = w_t.shape
    return np.ascontiguousarray(
        w_t.reshape(K // P, P, M).transpose(1, 0, 2)).astype(np.float32)


def kernel(embeddings, last_logits, Whx, bhx, Wyh, byh, Wout, bout,
           _T: int | None = None):
    T = T_FULL if _T is None else _T
    embeddings = np.asarray(embeddings, dtype=np.float32)[:, :T, :]
    last_logits = np.asarray(last_logits, dtype=np.float32)
    Whx = np.asarray(Whx, dtype=np.float32)
    bhx = np.asarray(bhx, dtype=np.float32)
    Wyh = np.asarray(Wyh, dtype=np.float32)
    byh = np.asarray(byh, dtype=np.float32)
    Wout = np.asarray(Wout, dtype=np.float32)
    bout = np.asarray(bout, dtype=np.float32)

    # Host-side constant folding (weights only).
    W2 = (Wyh.astype(np.float64) @ Wout.astype(np.float64)).astype(np.float32)
    c2 = (bout.astype(np.float64) @ Wyh.T.astype(np.float64)
          + byh.astype(np.float64)).astype(np.float32)

    w2t = _pack_kmaj(W2.T)                       # [128, 4, H]
    wyht = _pack_kmaj(np.ascontiguousarray(Wyh.T))
    whxt = _pack_kmaj(np.ascontiguousarray(Whx.T))
    woutt = _pack_kmaj(np.ascontiguousarray(Wout.T))
    xhb = np.ascontiguousarray(
        (bhx + c2).reshape(HJ, P).T).astype(np.float32)       # [128, 4]
    dcv = (byh - c2).reshape(HJ, P).T                         # [128, 4]
    dc = np.ascontiguousarray(
        np.repeat(dcv[:, :, None], BL, axis=2)).astype(np.float32)
    boutr = np.ascontiguousarray(
        np.broadcast_to(bout[None, :], (P, O))).astype(np.float32)

    nc = _get_nc(T)
    in_maps = []
    for c in range(NCORES):
        eb = embeddings[c * BL:(c + 1) * BL]          # [BL, T, D]
        y0 = last_logits[c * BL:(c + 1) * BL]         # [BL, O]
        y0t = np.ascontiguousarray(
            y0.T.reshape(HJ, P, BL).transpose(1, 0, 2)).astype(np.float32)
        in_maps.append({
            "emb": np.ascontiguousarray(eb.reshape(BL * T, D)),
            "y0t": y0t,
            "w2t": w2t, "wyht": wyht, "whxt": whxt, "woutt": woutt,
            "xhb": xhb, "dc": dc, "boutr": boutr,
        })

    res = run_bass_kernel_spmd(nc, in_maps, core_ids=list(range(NCORES)))
    globals()["LAST_RESULTS"] = res  # for external profiling harnesses
    logits = np.concatenate([r["logits"] for r in res.results], axis=0)
    return logits, np.ascontiguousarray(logits[:, -1, :])


# revision 10
# speedup vs baseline: 3.4137x; 3.4137x over previous
"""Trainium2 Bass kernel for the Jordan-RNN problem.

Math (reference):
    xh = emb @ Whx.T + bhx                      # [B,T,H] time-parallel
    scan t: h = tanh(xh_t + y @ Wyh.T + byh); y = h @ Wout.T + bout
    outputs: logits[:, t] = y_t ; last = y_T

Reformulation used here (y eliminated from the serial chain):
    W2 = Wyh @ Wout ; c2 = bout @ Wyh.T + byh
    xh2_t = emb_t @ Whx.T + bhx + c2            (phase 1, big GEMM)
    xh2_0 += y0 @ Wyh.T + byh - c2              (one-time device injection)
    h_t = tanh(xh2_t + h_{t-1} @ W2.T), h_{-1} = 0   (phase 2, serial)
    logits_t = h_t @ Wout.T + bout              (phase 3, big GEMM)

Sharding: data-parallel over batch across 8 cores (B_local = 8); weights
replicated; the T-recurrence stays local per core. SPMD: one NEFF, 8 cores.

Layouts (per core):
    emb      [BL*T, D]    natural
    xh2_d    [4, 128, BL, T]  (h-outer j, h-inner p, b, t)  == xh2^T tiles
    hall_d   [4, 128, BL, T]  same for h
    h in SBUF: [128, 4(j), 8(b)] per step -> directly the next step's matmul
    rhs (contraction dim H on partitions). No transposes in the recurrence.
"""

import os
import sys

sys.path.insert(0, "/opt/trn_rl_repo")

import ml_dtypes
import numpy as np

import concourse.bass as bass
import concourse.mybir as mybir
import concourse.tile as tile
from concourse import bacc
from concourse.bass import ds, ts
from concourse.bass_utils import run_bass_kernel_spmd
from concourse.masks import make_identity

F32 = mybir.dt.float32
BF16 = mybir.dt.bfloat16
P = 128
NCORES = 8
B, T_FULL, D, H, O = 64, 2048, 512, 512, 512
BL = B // NCORES  # 8
HJ = H // P       # 4 h-tiles
KD = D // P       # 4 d-tiles
U = 64            # steps per chunk
CHUNKS_PER_ITER = 2



def _tctile(tc, shape, dtype, **kw):
    t, _free = tc.tile(shape, dtype, **kw)
    return t

def _build(T: int):
    """Emit the full 3-phase kernel for sequence length T. Returns nc."""
    BT = BL * T
    nc = bacc.Bacc("TRN2", target_bir_lowering=False, debug=False)

    # ---- I/O ----
    emb = nc.dram_tensor("emb", [BT, D], F32, kind="ExternalInput").ap()
    y0t = nc.dram_tensor("y0t", [P, HJ, BL], F32, kind="ExternalInput").ap()
    w2t = nc.dram_tensor("w2t", [P, HJ, H], BF16, kind="ExternalInput").ap()
    wyht = nc.dram_tensor("wyht", [P, HJ, H], F32, kind="ExternalInput").ap()
    whxt = nc.dram_tensor("whxt", [P, KD, H], F32, kind="ExternalInput").ap()
    woutt = nc.dram_tensor("woutt", [P, HJ, O], BF16, kind="ExternalInput").ap()
    xhb = nc.dram_tensor("xhb", [P, HJ], F32, kind="ExternalInput").ap()
    dc = nc.dram_tensor("dc", [P, HJ, BL], F32, kind="ExternalInput").ap()
    boutr = nc.dram_tensor("boutr", [P, O], F32, kind="ExternalInput").ap()
    logits = nc.dram_tensor("logits", [BL, T, O], F32, kind="ExternalOutput").ap()

    with tile.TileContext(nc) as tc, \
         tc.tile_pool(name="cst", bufs=1) as cst:
        # ---- persistent SBUF constants ----
        w2t_sb = cst.tile([P, HJ, H], BF16, tag="w2t_sb")
        wyht_sb = cst.tile([P, HJ, H], F32, tag="wyht_sb")
        whxt_sb = cst.tile([P, KD, H], F32, tag="whxt_sb")
        woutt_sb = cst.tile([P, HJ, O], BF16, tag="woutt_sb")
        y0t_sb = cst.tile([P, HJ, BL], F32, tag="y0t_sb")
        xhb_sb = cst.tile([P, HJ], F32, tag="xhb_sb")
        dc_sb = cst.tile([P, HJ, BL], F32, tag="dc_sb")
        bout_sb = cst.tile([P, O], F32, tag="bout_sb")
        ident = cst.tile([P, P], F32, tag="ident")

        nc.sync.dma_start(w2t_sb, w2t)
        nc.sync.dma_start(wyht_sb, wyht)
        nc.sync.dma_start(whxt_sb, whxt)
        nc.sync.dma_start(woutt_sb, woutt)
        nc.sync.dma_start(y0t_sb, y0t)
        nc.sync.dma_start(xhb_sb, xhb)
        nc.sync.dma_start(dc_sb, dc)
        nc.sync.dma_start(bout_sb, boutr)
        make_identity(nc, ident)

        # ---- DRAM scratch (Tile-tracked) ----
        xh2_d = _tctile(tc, [P, HJ, BL, T], F32, space="DRAM", name="xh2_d")
        hall_d = _tctile(tc, [P, HJ, BL, T], BF16, space="DRAM", name="hall_d")

        # ================= Phase 1: xh2 = emb @ Whx.T + (bhx + c2) ========
        CS = min(512, T)   # bt-chunk size (within one b)
        SN = CS // P       # row-subtiles per chunk
        NCH = BT // CS
        TCH = T // CS      # t-chunks per b
        with tc.tile_pool(name="p1", bufs=3) as p1, \
             tc.tile_pool(name="p1pt", bufs=2, space="PSUM") as p1pt, \
             tc.tile_pool(name="p1pg", bufs=2, space="PSUM") as p1pg:
            for c in range(NCH):
                b, t0 = c // TCH, (c % TCH) * CS
                emb_nat = p1.tile([P, SN, 512], F32, tag="embnat")
                nc.sync.dma_start(
                    emb_nat,
                    emb[c * CS:(c + 1) * CS, :].rearrange(
                        "(s p) d -> p s d", p=P),
                )
                embT = p1.tile([P, KD, CS], F32, tag="embt")
                for kd in range(KD):
                    for s in range(SN):
                        pt = p1pt.tile([P, P], F32, tag="pt")
                        nc.tensor.transpose(
                            pt, emb_nat[:, s, kd * P:(kd + 1) * P], ident)
                        nc.vector.tensor_copy(
                            embT[:, kd, s * P:(s + 1) * P], pt)
                for j in range(HJ):
                    pg = p1pg.tile([P, CS], F32, tag="pg")
                    for kd in range(KD):
                        nc.tensor.matmul(
                            pg, whxt_sb[:, kd, ts(j, P)], embT[:, kd, :],
                            start=(kd == 0), stop=(kd == KD - 1))
                    xo = p1.tile([P, CS], F32, tag="xo")
                    nc.scalar.activation(
                        xo, pg, mybir.ActivationFunctionType.Identity,
                        bias=xhb_sb[:, j:j + 1])
                    nc.sync.dma_start(xh2_d[:, j, b, t0:t0 + CS], xo)

            # ---- inject Delta0 = y0 @ Wyh.T + byh - c2 into xh2[:, 0] ----
            pd = p1pg.tile([P, HJ * BL], F32, tag="pg0")
            for j in range(HJ):
                for k in range(HJ):
                    nc.tensor.matmul(
                        pd[:, ts(j, BL)], wyht_sb[:, k, ts(j, P)],
                        y0t_sb[:, k, :],
                        start=(j == 0 and k == 0),
                        stop=(j == HJ - 1 and k == HJ - 1))
            dsb = p1.tile([P, HJ, BL], F32, tag="dsb")
            nc.vector.tensor_add(
                dsb, pd.rearrange("p (j b) -> p j b", j=HJ), dc_sb)
            with nc.allow_non_contiguous_dma(reason="one-time 16KB inject"):
                nc.gpsimd.dma_start(
                    xh2_d[:, :, :, 0], dsb, accum_op=mybir.AluOpType.add)

        # ================= Phase 2: the T-step recurrence =================
        STEP = CHUNKS_PER_ITER * U
        with tc.tile_pool(name="rec", bufs=1) as rec, \
             tc.tile_pool(name="rpz", bufs=2, space="PSUM") as rpz:
            xh_t = [rec.tile([P, HJ, BL, U], F32, name=f"xh_{i}",
                             tag=f"xh_{i}") for i in range(CHUNKS_PER_ITER)]
            h_t = [rec.tile([P, HJ, BL, U], BF16, name=f"h_{i}",
                            tag=f"h_{i}") for i in range(CHUNKS_PER_ITER)]
            nc.vector.memset(h_t[-1], 0.0)  # h_{-1} = 0 seed (last slot used)

            with tc.For_i(0, T, STEP,
                          hint_engines=(mybir.EngineType.PE,)) as iv:
                for half in range(CHUNKS_PER_ITER):
                    xh_c = xh_t[half]
                    h_c = h_t[half]
                    h_p = h_t[half - 1]
                    base = iv + half * U
                    nc.sync.dma_start(
                        xh_c, xh2_d[:, :, :, ds(base, U)])
                    for u in range(U):
                        hp, up = (h_c, u - 1) if u > 0 else (h_p, U - 1)
                        pz = rpz.tile([P, HJ * BL], F32, tag="pz")
                        for j in range(HJ):
                            for k in range(HJ):
                                nc.tensor.matmul(
                                    pz[:, ts(j, BL)],
                                    w2t_sb[:, k, ts(j, P)],
                                    hp[:, k, :, up],
                                    start=(j == 0 and k == 0),
                                    stop=(j == HJ - 1 and k == HJ - 1))
                        us = rec.tile([P, HJ, BL], F32, tag="us", bufs=2)
                        nc.vector.tensor_add(
                            us, pz.rearrange("p (j b) -> p j b", j=HJ),
                            xh_c[:, :, :, u])
                        nc.scalar.activation(
                            h_c[:, :, :, u], us,
                            mybir.ActivationFunctionType.Tanh)
                    nc.sync.dma_start(
                        hall_d[:, :, :, ds(base, U)], h_c)

        # ================= Phase 3: logits = Hall @ Wout.T + bout =========
        TT = T // P
        with tc.tile_pool(name="p3", bufs=8) as p3, \
             tc.tile_pool(name="p3pg", bufs=2, space="PSUM") as p3pg, \
             tc.tile_pool(name="p3o", bufs=3) as p3o:
            for b in range(BL):
                for tt in range(TT):
                    pg = p3pg.tile([P, O], F32, tag="p3g")
                    for j in range(HJ):
                        lt = p3.tile([P, P], BF16, tag="lt")
                        nc.sync.dma_start(lt, hall_d[:, j, b, ts(tt, P)])
                        nc.tensor.matmul(
                            pg, lt, woutt_sb[:, j, :],
                            start=(j == 0), stop=(j == HJ - 1))
                    ob = p3o.tile([P, O], F32, tag="ob")
                    nc.vector.tensor_add(ob, pg, bout_sb)
                    nc.sync.dma_start(logits[b, ts(tt, P), :], ob)

    nc.compile()
    return nc


_CACHE: dict = {}


def _get_nc(T: int):
    if T not in _CACHE:
        _CACHE[T] = _build(T)
    return _CACHE[T]


def _pack_kmaj(w_t: np.ndarray) -> np.ndarray:
    """[K, M] (K = contraction, multiple of 128) -> [128, K//128, M]."""
    K, M = w_t.shape
    return np.ascontiguousarray(
        w_t.reshape(K // P, P, M).transpose(1, 0, 2)).astype(np.float32)


def kernel(embeddings, last_logits, Whx, bhx, Wyh, byh, Wout, bout,
           _T: int | None = None):
    T = T_FULL if _T is None else _T
    embeddings = np.asarray(embeddings, dtype=np.float32)[:, :T, :]
    last_logits = np.asarray(last_logits, dtype=np.float32)
    Whx = np.asarray(Whx, dtype=np.float32)
    bhx = np.asarray(bhx, dtype=np.float32)
    Wyh = np.asarray(Wyh, dtype=np.float32)
    byh = np.asarray(byh, dtype=np.float32)
    Wout = np.asarray(Wout, dtype=np.float32)
    bout = np.asarray(bout, dtype=np.float32)

    # Host-side constant folding (weights only).
    W2 = (Wyh.astype(np.float64) @ Wout.astype(np.float64)).astype(np.float32)
    c2 = (bout.astype(np.float64) @ Wyh.T.astype(np.float64)
          + byh.astype(np.float64)).astype(np.float32)

    w2t = _pack_kmaj(W2.T).astype(ml_dtypes.bfloat16)   # [128, 4, H]
    wyht = _pack_kmaj(np.ascontiguousarray(Wyh.T))
    whxt = _pack_kmaj(np.ascontiguousarray(Whx.T))
    woutt = _pack_kmaj(np.ascontiguousarray(Wout.T)).astype(ml_dtypes.bfloat16)
    xhb = np.ascontiguousarray(
        (bhx + c2).reshape(HJ, P).T).astype(np.float32)       # [128, 4]
    dcv = (byh - c2).reshape(HJ, P).T                         # [128, 4]
    dc = np.ascontiguousarray(
        np.repeat(dcv[:, :, None], BL, axis=2)).astype(np.float32)
    boutr = np.ascontiguousarray(
        np.broadcast_to(bout[None, :], (P, O))).astype(np.float32)

    nc = _get_nc(T)
    in_maps = []
    for c in range(NCORES):
        eb = embeddings[c * BL:(c + 1) * BL]          # [BL, T, D]
        y0 = last_logits[c * BL:(c + 1) * BL]         # [BL, O]
        y0t = np.ascontiguousarray(
            y0.T.reshape(HJ, P, BL).transpose(1, 0, 2)).astype(np.float32)
        in_maps.append({
            "emb": np.ascontiguousarray(eb.reshape(BL * T, D)),
            "y0t": y0t,
            "w2t": w2t, "wyht": wyht, "whxt": whxt, "woutt": woutt,
            "xhb": xhb, "dc": dc, "boutr": boutr,
        })

    res = run_bass_kernel_spmd(nc, in_maps, core_ids=list(range(NCORES)))
    globals()["LAST_RESULTS"] = res  # for external profiling harnesses
    logits = np.concatenate([r["logits"] for r in res.results], axis=0)
    return logits, np.ascontiguousarray(logits[:, -1, :])
